# revision 6
# baseline (speedup 1.0000x reference)
"""GRU block kernel for Trainium2, 8 NeuronCores, data-parallel over batch.

Problem: x[128,512,1629] f32, W_g[1757,128] (g in r,u,c), b_g[128].
  xproj_g = x @ W_g[128:] + b_g          (big memory-bound GEMM)
  recurrence over T=512:
     r = sigmoid(h @ Wh_r + xr_t); u = sigmoid(h @ Wh_u + xu_t)
     c = tanh((r*h) @ Wh_c + xc_t); h' = (1-u)*h + u*c
Output y[128,512,128] = h_t for all t.

Strategy per core (B_local=16):
 - host pre-transposes x slice to K-major layout xt[13,128,8192] (k-blocks,
   k-in-block, m) with m = t*16+b, zero-padded K 1629->1664.
 - GEMM phase: xprojT[h,g,m] accumulated in PSUM over 13 k-blocks, evicted
   via ScalarE Identity+bias (free bias add) into a resident SBUF buffer.
 - Recurrence in transposed layout (H=128 on partitions, batch on free dim),
   one chain; h state lives directly in the y output ring (SBUF), streamed
   to DRAM every 64 steps. Tile scheduler overlaps GEMM with recurrence.
"""

import numpy as np
from contextlib import ExitStack

import concourse.bass as bass
import concourse.bacc as bacc
import concourse.tile as tile
from concourse import mybir
from concourse import bass_utils

F32 = mybir.dt.float32
AF = mybir.ActivationFunctionType

B, T, K, H = 128, 512, 1629, 128
NC = 8
BL = B // NC          # 16 batch per core
M = T * BL            # 8192 flattened (t, b) per core
NKB = 13              # k-blocks of 128 (1664 padded)
KP = NKB * 128
MC = 512              # gemm m-chunk (one PSUM bank)
NMC = M // MC         # 16
SEG = 64              # recurrence steps per y ring segment


def build_program(t_steps=T, nmc=NMC):
    """Build the SPMD Bass program. Returns (nc, names dict)."""
    m = nmc * MC
    nc = bacc.Bacc("TRN2", target_bir_lowering=False, debug=False,
                   num_devices=NC)
    xt = nc.dram_tensor("xt", [NKB, 128, m], F32, kind="ExternalInput").ap()
    wx = nc.dram_tensor("wx", [3, NKB, 128, H], F32, kind="ExternalInput").ap()
    wh = nc.dram_tensor("wh", [3, H, H], F32, kind="ExternalInput").ap()
    bz = nc.dram_tensor("bz", [3, H, 1], F32, kind="ExternalInput").ap()
    y = nc.dram_tensor("y", [H, m], F32, kind="ExternalOutput").ap()

    with tile.TileContext(nc) as tc, ExitStack() as ctx:
        consts = ctx.enter_context(tc.tile_pool(name="consts", bufs=1))
        xpp = ctx.enter_context(tc.tile_pool(name="xproj", bufs=1))
        xpool = ctx.enter_context(tc.tile_pool(name="xtiles", bufs=2))
        gpsum = ctx.enter_context(tc.tile_pool(name="gpsum", bufs=2, space="PSUM"))
        rupsum = ctx.enter_context(tc.tile_pool(name="rupsum", bufs=2, space="PSUM"))
        cpsum = ctx.enter_context(tc.tile_pool(name="cpsum", bufs=2, space="PSUM"))
        ypool = ctx.enter_context(tc.tile_pool(name="yring", bufs=2))
        small = ctx.enter_context(tc.tile_pool(name="small", bufs=4))
        state = ctx.enter_context(tc.tile_pool(name="state", bufs=1))

        # ---- load constants ----
        wxt = [[consts.tile([128, H], F32, name=f"wx{g}_{kb}", tag=f"wx{g}_{kb}")
                for kb in range(NKB)] for g in range(3)]
        wht = [consts.tile([H, H], F32, name=f"wh{g}", tag=f"wh{g}") for g in range(3)]
        bzt = [consts.tile([H, 1], F32, name=f"bz{g}", tag=f"bz{g}") for g in range(3)]
        for g in range(3):
            for kb in range(NKB):
                nc.sync.dma_start(out=wxt[g][kb], in_=wx[g, kb])
            nc.sync.dma_start(out=wht[g], in_=wh[g])
            nc.sync.dma_start(out=bzt[g], in_=bz[g])

        # resident xproj buffer [128, 3, m]
        xp = xpp.tile([128, 3, m], F32, name="xp", tag="xp")

        # ---- GEMM phase: xprojT = Wx_g.T @ xT (+bias on evict) ----
        for mc in range(nmc):
            xtile = xpool.tile([128, NKB, MC], F32, name="xtile", tag="xtile")
            nc.sync.dma_start(
                out=xtile,
                in_=xt[:, :, mc * MC:(mc + 1) * MC].rearrange("k p m -> p k m"))
            for g in range(3):
                ps = gpsum.tile([128, MC], F32, name="gps", tag="gps")
                for kb in range(NKB):
                    nc.tensor.matmul(ps, lhsT=wxt[g][kb],
                                     rhs=xtile[:, kb, :],
                                     start=(kb == 0), stop=(kb == NKB - 1))
                # evict with free per-partition bias add
                nc.scalar.add(xp[:, g, mc * MC:(mc + 1) * MC], ps, add=bzt[g])

        # ---- recurrence ----
        h_prev = state.tile([128, BL], F32, name="h0", tag="h0")
        nc.vector.memset(h_prev, 0.0)
        nseg = (t_steps + SEG - 1) // SEG
        for seg in range(nseg):
            steps = min(SEG, t_steps - seg * SEG)
            yseg = ypool.tile([128, SEG * BL], F32, name="yseg", tag="yseg")
            for tt in range(steps):
                t = seg * SEG + tt
                ts = t * BL
                col = tt * BL
                p_ru = rupsum.tile([128, 2 * BL], F32, name="pru", tag="pru")
                p_c = cpsum.tile([128, BL], F32, name="pc", tag="pc")
                nc.tensor.matmul(p_ru[:, 0:BL], lhsT=wht[0], rhs=h_prev,
                                 start=True, stop=True)
                nc.tensor.matmul(p_ru[:, BL:2 * BL], lhsT=wht[1],
                                 rhs=h_prev, start=True, stop=True)
                pru3 = p_ru.rearrange("p (g b) -> p g b", g=2)
                nc.vector.tensor_add(pru3, pru3, xp[:, 0:2, ts:ts + BL])
                ru = small.tile([128, 2 * BL], F32, name="ru", tag="ru")
                nc.scalar.activation(ru, p_ru, AF.Sigmoid)
                t1 = small.tile([128, BL], F32, name="t1", tag="t1")
                nc.vector.tensor_mul(t1, ru[:, 0:BL], h_prev)
                nc.tensor.matmul(p_c, lhsT=wht[2], rhs=t1,
                                 start=True, stop=True)
                nc.vector.tensor_add(p_c, p_c, xp[:, 2, ts:ts + BL])
                c_t = small.tile([128, BL], F32, name="ct", tag="ct")
                nc.scalar.activation(c_t, p_c, AF.Tanh)
                d = small.tile([128, BL], F32, name="d", tag="d")
                nc.vector.tensor_sub(d, c_t, h_prev)
                mm = small.tile([128, BL], F32, name="mm", tag="mm")
                nc.vector.tensor_mul(mm, ru[:, BL:2 * BL], d)
                h_new = yseg[:, col:col + BL]
                nc.vector.tensor_add(h_new, h_prev, mm)
                h_prev = h_new
            nc.sync.dma_start(
                out=y[:, seg * SEG * BL: seg * SEG * BL + steps * BL],
                in_=yseg[:, 0:steps * BL])

    nc.compile()
    return nc


def prep_inputs(x, W_r, b_r, W_u, b_u, W_c, b_c):
    """Host-side shard + layout transform. Returns in_maps list for 8 cores."""
    ws = [W_r, W_u, W_c]
    bs = [b_r, b_u, b_c]
    wx = np.zeros((3, NKB, 128, H), dtype=np.float32)
    wh = np.zeros((3, H, H), dtype=np.float32)
    bz = np.zeros((3, H, 1), dtype=np.float32)
    for g in range(3):
        wxa = np.zeros((KP, H), dtype=np.float32)
        wxa[:K] = ws[g][H:]
        wx[g] = wxa.reshape(NKB, 128, H)
        wh[g] = ws[g][:H]
        bz[g, :, 0] = bs[g]
    in_maps = []
    for c in range(NC):
        xs = x[c * BL:(c + 1) * BL]                 # [BL, T, K]
        xtc = np.zeros((KP, M), dtype=np.float32)
        # m = t*BL + b ; xt[k, m] = x[b, t, k]
        xtc[:K] = xs.transpose(2, 1, 0).reshape(K, M)
        in_maps.append({
            "xt": np.ascontiguousarray(xtc.reshape(NKB, 128, M)),
            "wx": wx, "wh": wh, "bz": bz,
        })
    return in_maps


_CACHED = {}


def kernel(x, W_r, b_r, W_u, b_u, W_c, b_c):
    if "nc" not in _CACHED:
        _CACHED["nc"] = build_program()
    nc = _CACHED["nc"]
    in_maps = prep_inputs(x, W_r, b_r, W_u, b_u, W_c, b_c)
    res = bass_utils.run_bass_kernel_spmd(
        nc, in_maps, core_ids=list(range(NC)), trace=False)
    _CACHED["last_results"] = res
    out = np.empty((B, T, H), dtype=np.float32)
    for c in range(NC):
        yc = res.results[c]["y"]                    # [H, M]
        out[c * BL:(c + 1) * BL] = yc.reshape(H, T, BL).transpose(2, 1, 0)
    return out


def timed_exec(x, W_r, b_r, W_u, b_u, W_c, b_c, iters=4):
    """Run the kernel with device-resident inputs, return (out, best_ns).

    Mirrors bass2jax.run_bass_via_pjrt's multi-core path but keeps inputs
    on device and times repeated executions (min over iters).
    """
    import time
    import jax
    from jax.sharding import Mesh, PartitionSpec, NamedSharding
    from jax.experimental.shard_map import shard_map
    from concourse import bass2jax, mybir as _mybir

    if "nc" not in _CACHED:
        _CACHED["nc"] = build_program()
    nc = _CACHED["nc"]
    bass2jax.install_neuronx_cc_hook()
    in_maps = prep_inputs(x, W_r, b_r, W_u, b_u, W_c, b_c)

    pname = nc.partition_id_tensor.name if nc.partition_id_tensor else None
    in_names, out_names, out_avals, zero_outs = [], [], [], []
    for alloc in nc.m.functions[0].allocations:
        if not isinstance(alloc, _mybir.MemoryLocationSet):
            continue
        name = alloc.memorylocations[0].name
        if alloc.kind == "ExternalInput":
            if name != pname:
                in_names.append(name)
        elif alloc.kind == "ExternalOutput":
            out_names.append(name)
            shape = tuple(alloc.tensor_shape)
            dtype = _mybir.dt.np(alloc.dtype)
            out_avals.append(jax.core.ShapedArray(shape, dtype))
            zero_outs.append(np.zeros(shape, dtype))
    n_params = len(in_names)
    all_names = in_names + out_names
    if pname is not None:
        all_names = all_names + [pname]

    def _body(*args):
        operands = list(args)
        if pname is not None:
            operands.append(bass2jax.partition_id_tensor())
        outs = bass2jax._bass_exec_p.bind(
            *operands, out_avals=tuple(out_avals), in_names=tuple(all_names),
            out_names=tuple(out_names), lowering_input_output_aliases=(),
            sim_require_finite=True, sim_require_nnan=True, nc=nc)
        return tuple(outs)

    devices = jax.devices()[:NC]
    mesh = Mesh(np.asarray(devices), ("core",))
    n_outs = len(out_names)
    sharded = jax.jit(
        shard_map(_body, mesh=mesh,
                  in_specs=(PartitionSpec("core"),) * (n_params + n_outs),
                  out_specs=(PartitionSpec("core"),) * n_outs,
                  check_rep=False),
        donate_argnums=tuple(range(n_params, n_params + n_outs)),
        keep_unused=True)

    sh = NamedSharding(mesh, PartitionSpec("core"))
    in_dev = [jax.device_put(
        np.concatenate([in_maps[c][nm] for c in range(NC)], axis=0), sh)
        for nm in in_names]
    zeros_np = [np.zeros((NC * z.shape[0], *z.shape[1:]), z.dtype)
                for z in zero_outs]

    best = None
    out_arrs = None
    for it in range(iters):
        zd = [jax.device_put(z, sh) for z in zeros_np]
        jax.block_until_ready(zd)
        t0 = time.perf_counter()
        out_arrs = sharded(*in_dev, *zd)
        jax.block_until_ready(out_arrs)
        dt = time.perf_counter() - t0
        print(f"  iter {it}: {dt*1e6:.0f} us", flush=True)
        if best is None or dt < best:
            best = dt

    out = np.empty((B, T, H), dtype=np.float32)
    yall = np.asarray(out_arrs[out_names.index("y")]).reshape(NC, H, M)
    for c in range(NC):
        out[c * BL:(c + 1) * BL] = yall[c].reshape(H, T, BL).transpose(2, 1, 0)
    return out, best * 1e9
